# revision 45
# baseline (speedup 1.0000x reference)
import sys

sys.path.insert(0, "/opt/trn_rl_repo")

import os
import numpy as np
import ml_dtypes

import concourse.bass as bass
import concourse.mybir as mybir
import concourse.tile as tile
from concourse import bacc
from concourse.bass_utils import run_bass_kernel_spmd
from concourse.masks import make_identity

B, S, D, H = 4, 4096, 1024, 64
QW = 512                      # q-chunk width
NQ = 4                        # q-chunk slots per core
POS = [(0, 3, 4, 7), (1, 2, 5, 6)]   # q-chunk positions per core class
ND = D // 128                 # 8 d-tiles
NC = S // QW                  # 8 chunks per batch
PAIR_BASE = [0, 4, 12, 24]    # exp-pair index base per slot
NPAIR = 40

BF = mybir.dt.bfloat16
F32 = mybir.dt.float32

_cache = {}


def _build():
    nc = bacc.Bacc("TRN2", target_bir_lowering=False, debug=False, num_devices=8)

    # xtr: per-core column-permuted x^T in chunk-major blocks: block (c, d) of
    # [128, 512] at cols (8c+d)*512. Chunk order: my0, other0, my1, other1, ...
    xtr = nc.dram_tensor("xtr", [128, NC * ND * 512], BF,
                         kind="ExternalInput").ap()
    # wpack: per d-tile: [Wq_d (64) | Wk_d (64) | Wv_d (64)] -> 192 cols each
    wpack = nc.dram_tensor("wpack", [128, ND * 192], BF, kind="ExternalInput").ap()
    # per-(slot,pair) exp bias column: 0.0 keep, -240.0 drop
    biast = nc.dram_tensor("biast", [128, NPAIR], F32, kind="ExternalInput").ap()
    o = nc.dram_tensor("o", [NQ, H + 1, QW], F32, kind="ExternalOutput").ap()

    with tile.TileContext(nc) as tc:
        with (
            tc.tile_pool(name="persist", bufs=1) as pp,
            tc.tile_pool(name="xin", bufs=1) as xp,
            tc.tile_pool(name="estage", bufs=3) as ep,
            tc.tile_pool(name="ostage", bufs=2) as osp,
            tc.tile_pool(name="zpsum", bufs=2, space="PSUM") as zp,
            tc.tile_pool(name="opsum", bufs=1, space="PSUM") as op_,
            tc.tile_pool(name="projpsum", bufs=2, space="PSUM") as prp,
            tc.tile_pool(name="vtpsum", bufs=1, space="PSUM") as vtp,
        ):
            # ---- persistent tiles ----
            w_sb = pp.tile([128, ND * 192], BF, tag="wpack")
            bias_sb = pp.tile([128, NPAIR], F32, tag="biast")
            ident = pp.tile([H + 1, H + 1], BF, tag="ident")
            qT = pp.tile([64, NQ * QW], BF, tag="qT")
            kT = pp.tile([64, S], BF, tag="kT")
            vws = pp.tile([128, S // 128, H + 2], BF, tag="vws")
            vstage = pp.tile([H + 1, 2, QW], BF, tag="vstage")
            xt_sb = xp.tile([128, NC, ND, 512], BF, tag="xt")

            def wq(d):
                return w_sb[:, d * 192:d * 192 + 64]

            def wkv(d):
                return w_sb[:, d * 192 + 64:(d + 1) * 192]

            # ---- input DMAs (issue order == arrival order) ----
            nc.sync.dma_start(w_sb[:], wpack[:])
            nc.sync.dma_start(bias_sb[:], biast[:])
            # chunk 0 split in 4 so multiple DMA engines run in parallel and
            # the first matmul can start sooner; later chunks as 1MB singles
            for p in range(4):
                nc.sync.dma_start(xt_sb[:, 0, 2 * p:2 * p + 2, :],
                                  xtr[:, p * 1024:(p + 1) * 1024])
            for c in range(1, NC):
                nc.sync.dma_start(xt_sb[:, c, :, :],
                                  xtr[:, c * ND * 512:(c + 1) * ND * 512])

            make_identity(nc, ident[:])
            # ones row for the softmax denominator (col 64 of every vws tile)
            nc.gpsimd.memset(vstage[64:65, :, :], 1.0)
            # causal masks for the two diagonal pairs of each slot, built once:
            # keep where q - k_local - 128*t >= 0 (t = tile within my chunk)
            mask_sb = pp.tile([128, 2, 1024], BF, tag="mask")
            nc.gpsimd.memset(mask_sb[:], 1.0)
            for m in range(2):
                nc.gpsimd.affine_select(
                    mask_sb[:, m, :], mask_sb[:, m, :],
                    pattern=[[-128, 2], [1, 512]],
                    compare_op=mybir.AluOpType.is_ge,
                    fill=0.0,
                    base=-256 * m,
                    channel_multiplier=-1)
            # warm the ACT exp table before the attention phase needs it
            warm = ep.tile([128, 1], BF, tag="warm")
            nc.scalar.activation(warm[:], bias_sb[:, 0:1],
                                 mybir.ActivationFunctionType.Exp)

            def qproj(s):
                ps = prp.tile([128, QW], F32, tag="proj", name=f"qps{s}")
                for d in range(ND):
                    nc.tensor.matmul(ps[0:64, :], wq(d),
                                     xt_sb[:, 2 * s, d, :],
                                     start=(d == 0), stop=(d == ND - 1))
                nc.vector.tensor_copy(qT[:, s * QW:(s + 1) * QW], ps[0:64, :])

            def kv_chunk(c):
                ps = prp.tile([128, QW], F32, tag="proj", name=f"kvps{c}")
                for d in range(ND):
                    nc.tensor.matmul(ps[:], wkv(d),
                                     xt_sb[:, c, d, :],
                                     start=(d == 0), stop=(d == ND - 1))
                nc.vector.tensor_copy(kT[:, c * 512:(c + 1) * 512], ps[0:64, :])
                vb = c & 1
                nc.vector.tensor_copy(vstage[0:64, vb, :], ps[64:128, :])
                vt = vtp.tile([128, 4, H + 2], BF, tag="vt", name=f"vt{c}")
                for t in range(4):
                    nc.tensor.transpose(vt[:, t, 0:65],
                                        vstage[:, vb, t * 128:(t + 1) * 128],
                                        ident[:])
                nc.vector.tensor_copy(vws[:, 4 * c:4 * c + 4, :], vt[:])

            def attn_pair(s, jj, start, stop):
                z = zp.tile([128, 1024], F32, tag="z", name=f"z{s}_{jj}")
                e = ep.tile([128, 1024], BF, tag="e", name=f"e{s}_{jj}")
                for h2 in range(2):
                    j = 2 * jj + h2
                    nc.tensor.matmul(z[:, h2 * 512:(h2 + 1) * 512],
                                     kT[:, j * 128:(j + 1) * 128],
                                     qT[:, s * 512:(s + 1) * 512],
                                     start=True, stop=True)
                col = PAIR_BASE[s] + jj
                nc.scalar.activation(e[:], z[:],
                                     mybir.ActivationFunctionType.Exp,
                                     bias=bias_sb[:, col:col + 1],
                                     scale=0.125)
                if jj == 4 * s or jj == 4 * s + 1:
                    # causal mask on my chunk's diagonal tiles
                    m = jj - 4 * s
                    nc.vector.tensor_tensor(e[:], e[:], mask_sb[:, m, :],
                                            mybir.AluOpType.mult)
                ops = _slot_ops[s]
                for h2 in range(2):
                    j = 2 * jj + h2
                    nc.tensor.matmul(ops[:],
                                     vws[:, j, 0:H + 1],
                                     e[:, h2 * 512:(h2 + 1) * 512],
                                     start=(start and h2 == 0),
                                     stop=(stop and h2 == 1))

            def slot_out(s):
                osb = osp.tile([H + 1, QW], F32, tag="osb", name=f"osb{s}")
                nc.vector.tensor_copy(osb[:], _slot_ops[s][:])
                nc.sync.dma_start(o[s], osb[:])

            _slot_ops = {}

            def attn_slot(s, fillers):
                _slot_ops[s] = op_.tile([H + 1, QW], F32, tag="oacc",
                                        name=f"oacc{s}")
                # diagonal pairs first so their exp+mask latency hides behind
                # the rest of the slot; bias-only pairs close the slot.
                seq = [4 * s, 4 * s + 1] + list(range(4 * s)) + [4 * s + 2,
                                                                 4 * s + 3]
                nf = len(fillers)
                for i, jj in enumerate(seq):
                    attn_pair(s, jj, start=(i == 0), stop=(i == len(seq) - 1))
                    if i < nf:
                        fillers[i]()
                for f in fillers[len(seq):]:
                    f()
                slot_out(s)

            qproj(0)
            kv_chunk(0)
            kv_chunk(1)
            attn_slot(0, [lambda: kv_chunk(2), lambda: kv_chunk(3),
                          lambda: qproj(1)])
            attn_slot(1, [lambda: kv_chunk(4), lambda: kv_chunk(5),
                          lambda: qproj(2)])
            attn_slot(2, [lambda: kv_chunk(6), lambda: kv_chunk(7),
                          lambda: qproj(3)])
            attn_slot(3, [])

    nc.compile()
    return nc


def _get_nc():
    if "nc" not in _cache:
        _cache["nc"] = _build()
    return _cache["nc"]


def kernel(x, Wk, Wq, Wv):
    x = np.asarray(x, dtype=np.float32)
    Wk = np.asarray(Wk, dtype=np.float32)
    Wq = np.asarray(Wq, dtype=np.float32)
    Wv = np.asarray(Wv, dtype=np.float32)

    nc = _get_nc()

    wkv_np = np.concatenate([Wk, Wv], axis=1)  # [1024, 128]
    wpack_np = np.zeros((128, ND * 192), np.float32)
    for d in range(ND):
        wpack_np[:, d * 192:d * 192 + 64] = Wq[d * 128:(d + 1) * 128, :]
        wpack_np[:, d * 192 + 64:(d + 1) * 192] = wkv_np[d * 128:(d + 1) * 128, :]
    wpack_np = wpack_np.astype(ml_dtypes.bfloat16)

    xt_b = [np.ascontiguousarray(x[b].T).astype(ml_dtypes.bfloat16)
            for b in range(B)]

    in_maps = []
    for c in range(8):
        b, cls = c >> 1, c & 1
        mine, other = POS[cls], POS[1 - cls]
        seq = []
        for s in range(NQ):
            seq.append(mine[s])
            seq.append(other[s])
        blocks = []
        for ch in seq:
            for d in range(ND):
                blocks.append(xt_b[b][d * 128:(d + 1) * 128,
                                      ch * QW:(ch + 1) * QW])
        xtr_np = np.ascontiguousarray(np.concatenate(blocks, axis=1))

        bias_np = np.zeros((128, NPAIR), np.float32)
        for s in range(NQ):
            if other[s] > mine[s]:
                bias_np[:, PAIR_BASE[s] + 4 * s + 2] = -240.0
                bias_np[:, PAIR_BASE[s] + 4 * s + 3] = -240.0

        in_maps.append({
            "xtr": xtr_np,
            "wpack": wpack_np,
            "biast": bias_np,
        })

    trace = bool(int(os.environ.get("KERNEL_TRACE", "0")))
    res = run_bass_kernel_spmd(nc, in_maps, core_ids=list(range(8)), trace=trace)
    _cache["last_result"] = res

    out = np.zeros((B, S, H), np.float32)
    for c in range(8):
        b, cls = c >> 1, c & 1
        oc = res.results[c]["o"]          # [NQ, 65, 512]
        for s, p in enumerate(POS[cls]):
            num = oc[s, 0:H, :]           # [64, 512]
            den = oc[s, H, :]             # [512]
            out[b, p * QW:(p + 1) * QW, :] = (num / den[None, :]).T
    return out


# revision 47
# speedup vs baseline: 1.1998x; 1.1998x over previous
import sys

sys.path.insert(0, "/opt/trn_rl_repo")

import os
import numpy as np
import ml_dtypes

import concourse.bass as bass
import concourse.mybir as mybir
import concourse.tile as tile
from concourse import bacc
from concourse.bass_utils import run_bass_kernel_spmd
from concourse.masks import make_identity

B, S, D, H = 4, 4096, 1024, 64
QW = 512                      # q-chunk width
NQ = 4                        # q-chunk slots per core
POS = [(0, 3, 4, 7), (1, 2, 5, 6)]   # q-chunk positions per core class
ND = D // 128                 # 8 d-tiles
NC = S // QW                  # 8 chunks per batch
PAIR_BASE = [0, 4, 12, 24]    # exp-pair index base per slot
NPAIR = 40

BF = mybir.dt.bfloat16
F32 = mybir.dt.float32

_cache = {}


def _build():
    nc = bacc.Bacc("TRN2", target_bir_lowering=False, debug=False, num_devices=8)

    # xtr: per-core column-permuted x^T in chunk-major blocks: block (c, d) of
    # [128, 512] at cols (8c+d)*512. Chunk order: my0, other0, my1, other1, ...
    xtr = nc.dram_tensor("xtr", [128, NC * ND * 512], BF,
                         kind="ExternalInput").ap()
    # wpack: per d-tile: [Wq_d (64) | Wk_d (64) | Wv_d (64)] -> 192 cols each
    wpack = nc.dram_tensor("wpack", [128, ND * 192], BF, kind="ExternalInput").ap()
    # per-(slot,pair) exp bias column: 0.0 keep, -240.0 drop
    biast = nc.dram_tensor("biast", [128, NPAIR], F32, kind="ExternalInput").ap()
    o = nc.dram_tensor("o", [NQ, H + 1, QW], F32, kind="ExternalOutput").ap()

    with tile.TileContext(nc) as tc:
        with (
            tc.tile_pool(name="persist", bufs=1) as pp,
            tc.tile_pool(name="xin", bufs=1) as xp,
            tc.tile_pool(name="estage", bufs=3) as ep,
            tc.tile_pool(name="ostage", bufs=2) as osp,
            tc.tile_pool(name="zpsum", bufs=2, space="PSUM") as zp,
            tc.tile_pool(name="opsum", bufs=1, space="PSUM") as op_,
            tc.tile_pool(name="projpsum", bufs=2, space="PSUM") as prp,
            tc.tile_pool(name="vtpsum", bufs=1, space="PSUM") as vtp,
        ):
            # ---- persistent tiles ----
            w_sb = pp.tile([128, ND * 192], BF, tag="wpack")
            bias_sb = pp.tile([128, NPAIR], F32, tag="biast")
            ident = pp.tile([H + 1, H + 1], BF, tag="ident")
            qT = pp.tile([64, NQ * QW], BF, tag="qT")
            kT = pp.tile([64, S], BF, tag="kT")
            vws = pp.tile([128, S // 128, H + 2], BF, tag="vws")
            vstage = pp.tile([H + 1, 2, QW], BF, tag="vstage")
            xt_sb = xp.tile([128, NC, ND, 512], BF, tag="xt")

            def wq(d):
                return w_sb[:, d * 192:d * 192 + 64]

            def wkv(d):
                return w_sb[:, d * 192 + 64:(d + 1) * 192]

            # ---- input DMAs (issue order == arrival order) ----
            nc.sync.dma_start(w_sb[:], wpack[:])
            # chunk 0 split per d-tile and chunk 1 in halves so many DMA
            # engines run in parallel and the first matmuls start sooner;
            # later chunks as 1MB singles
            for p in range(8):
                nc.sync.dma_start(xt_sb[:, 0, p, :],
                                  xtr[:, p * 512:(p + 1) * 512])
            nc.sync.dma_start(bias_sb[:], biast[:])
            for p in range(2):
                nc.sync.dma_start(xt_sb[:, 1, 4 * p:4 * p + 4, :],
                                  xtr[:, (8 + 4 * p) * 512:(12 + 4 * p) * 512])
            for c in range(2, NC):
                nc.sync.dma_start(xt_sb[:, c, :, :],
                                  xtr[:, c * ND * 512:(c + 1) * ND * 512])

            make_identity(nc, ident[:])
            # ones row for the softmax denominator (col 64 of every vws tile)
            nc.gpsimd.memset(vstage[64:65, :, :], 1.0)
            # causal masks for the two diagonal pairs of each slot, built once:
            # keep where q - k_local - 128*t >= 0 (t = tile within my chunk)
            mask_sb = pp.tile([128, 2, 1024], BF, tag="mask")
            nc.gpsimd.memset(mask_sb[:], 1.0)
            for m in range(2):
                nc.gpsimd.affine_select(
                    mask_sb[:, m, :], mask_sb[:, m, :],
                    pattern=[[-128, 2], [1, 512]],
                    compare_op=mybir.AluOpType.is_ge,
                    fill=0.0,
                    base=-256 * m,
                    channel_multiplier=-1)
            # warm the ACT exp table before the attention phase needs it
            warm = ep.tile([128, 1], BF, tag="warm")
            nc.scalar.activation(warm[:], bias_sb[:, 0:1],
                                 mybir.ActivationFunctionType.Exp)

            def qproj(s):
                ps = prp.tile([128, QW], F32, tag="proj", name=f"qps{s}")
                for d in range(ND):
                    nc.tensor.matmul(ps[0:64, :], wq(d),
                                     xt_sb[:, 2 * s, d, :],
                                     start=(d == 0), stop=(d == ND - 1))
                nc.vector.tensor_copy(qT[:, s * QW:(s + 1) * QW], ps[0:64, :])

            def kv_chunk(c):
                ps = prp.tile([128, QW], F32, tag="proj", name=f"kvps{c}")
                for d in range(ND):
                    nc.tensor.matmul(ps[:], wkv(d),
                                     xt_sb[:, c, d, :],
                                     start=(d == 0), stop=(d == ND - 1))
                nc.vector.tensor_copy(kT[:, c * 512:(c + 1) * 512], ps[0:64, :])
                vb = c & 1
                nc.vector.tensor_copy(vstage[0:64, vb, :], ps[64:128, :])
                vt = vtp.tile([128, 4, H + 2], BF, tag="vt", name=f"vt{c}")
                for t in range(4):
                    nc.tensor.transpose(vt[:, t, 0:65],
                                        vstage[:, vb, t * 128:(t + 1) * 128],
                                        ident[:])
                nc.vector.tensor_copy(vws[:, 4 * c:4 * c + 4, :], vt[:])

            def attn_pair(s, jj, start, stop):
                z = zp.tile([128, 1024], F32, tag="z", name=f"z{s}_{jj}")
                e = ep.tile([128, 1024], BF, tag="e", name=f"e{s}_{jj}")
                for h2 in range(2):
                    j = 2 * jj + h2
                    nc.tensor.matmul(z[:, h2 * 512:(h2 + 1) * 512],
                                     kT[:, j * 128:(j + 1) * 128],
                                     qT[:, s * 512:(s + 1) * 512],
                                     start=True, stop=True)
                col = PAIR_BASE[s] + jj
                nc.scalar.activation(e[:], z[:],
                                     mybir.ActivationFunctionType.Exp,
                                     bias=bias_sb[:, col:col + 1],
                                     scale=0.125)
                if jj == 4 * s or jj == 4 * s + 1:
                    # causal mask on my chunk's diagonal tiles
                    m = jj - 4 * s
                    nc.vector.tensor_tensor(e[:], e[:], mask_sb[:, m, :],
                                            mybir.AluOpType.mult)
                ops = _slot_ops[s]
                for h2 in range(2):
                    j = 2 * jj + h2
                    nc.tensor.matmul(ops[:],
                                     vws[:, j, 0:H + 1],
                                     e[:, h2 * 512:(h2 + 1) * 512],
                                     start=(start and h2 == 0),
                                     stop=(stop and h2 == 1))

            def slot_out(s):
                osb = osp.tile([H + 1, QW], F32, tag="osb", name=f"osb{s}")
                nc.vector.tensor_copy(osb[:], _slot_ops[s][:])
                nc.sync.dma_start(o[s], osb[:])

            _slot_ops = {}

            def attn_slot(s, fillers):
                _slot_ops[s] = op_.tile([H + 1, QW], F32, tag="oacc",
                                        name=f"oacc{s}")
                # diagonal pairs first so their exp+mask latency hides behind
                # the rest of the slot; bias-only pairs close the slot.
                seq = [4 * s, 4 * s + 1] + list(range(4 * s)) + [4 * s + 2,
                                                                 4 * s + 3]
                nf = len(fillers)
                for i, jj in enumerate(seq):
                    attn_pair(s, jj, start=(i == 0), stop=(i == len(seq) - 1))
                    if i < nf:
                        fillers[i]()
                for f in fillers[len(seq):]:
                    f()
                slot_out(s)

            qproj(0)
            kv_chunk(0)
            kv_chunk(1)
            attn_slot(0, [lambda: kv_chunk(2), lambda: kv_chunk(3),
                          lambda: qproj(1)])
            attn_slot(1, [lambda: kv_chunk(4), lambda: kv_chunk(5),
                          lambda: qproj(2)])
            attn_slot(2, [lambda: kv_chunk(6), lambda: kv_chunk(7),
                          lambda: qproj(3)])
            attn_slot(3, [])

    nc.compile()
    return nc


def _get_nc():
    if "nc" not in _cache:
        _cache["nc"] = _build()
    return _cache["nc"]


def kernel(x, Wk, Wq, Wv):
    x = np.asarray(x, dtype=np.float32)
    Wk = np.asarray(Wk, dtype=np.float32)
    Wq = np.asarray(Wq, dtype=np.float32)
    Wv = np.asarray(Wv, dtype=np.float32)

    nc = _get_nc()

    wkv_np = np.concatenate([Wk, Wv], axis=1)  # [1024, 128]
    wpack_np = np.zeros((128, ND * 192), np.float32)
    for d in range(ND):
        wpack_np[:, d * 192:d * 192 + 64] = Wq[d * 128:(d + 1) * 128, :]
        wpack_np[:, d * 192 + 64:(d + 1) * 192] = wkv_np[d * 128:(d + 1) * 128, :]
    wpack_np = wpack_np.astype(ml_dtypes.bfloat16)

    xt_b = [np.ascontiguousarray(x[b].T).astype(ml_dtypes.bfloat16)
            for b in range(B)]

    in_maps = []
    for c in range(8):
        b, cls = c >> 1, c & 1
        mine, other = POS[cls], POS[1 - cls]
        seq = []
        for s in range(NQ):
            seq.append(mine[s])
            seq.append(other[s])
        blocks = []
        for ch in seq:
            for d in range(ND):
                blocks.append(xt_b[b][d * 128:(d + 1) * 128,
                                      ch * QW:(ch + 1) * QW])
        xtr_np = np.ascontiguousarray(np.concatenate(blocks, axis=1))

        bias_np = np.zeros((128, NPAIR), np.float32)
        for s in range(NQ):
            if other[s] > mine[s]:
                bias_np[:, PAIR_BASE[s] + 4 * s + 2] = -240.0
                bias_np[:, PAIR_BASE[s] + 4 * s + 3] = -240.0

        in_maps.append({
            "xtr": xtr_np,
            "wpack": wpack_np,
            "biast": bias_np,
        })

    trace = bool(int(os.environ.get("KERNEL_TRACE", "0")))
    res = run_bass_kernel_spmd(nc, in_maps, core_ids=list(range(8)), trace=trace)
    _cache["last_result"] = res

    out = np.zeros((B, S, H), np.float32)
    for c in range(8):
        b, cls = c >> 1, c & 1
        oc = res.results[c]["o"]          # [NQ, 65, 512]
        for s, p in enumerate(POS[cls]):
            num = oc[s, 0:H, :]           # [64, 512]
            den = oc[s, H, :]             # [512]
            out[b, p * QW:(p + 1) * QW, :] = (num / den[None, :]).T
    return out
